# revision 9
# baseline (speedup 1.0000x reference)
"""Trainium2 Bass kernel for nn_NewModel_66176856097442 (TransE-style loss).

Strategy (data-parallel over the batch of triples, v2 — dma_gather two-stage):
  - B = 262144 triples sharded as 32768/core across 8 NeuronCores; the fp16
    vec table [100000, 128] (256B rows) is replicated per core.
  - Per chunk of C=4096 triples, the 4*C entity-row fetches are compacted by
    entity bucket (4 buckets of 32768 rows so indices fit the SWDGE gather
    ucode's int16 limit) and fetched with 4 bulk `dma_gather` calls into an
    SBUF pool — ~1us + 0.34ns/row per call instead of ~1us per 128 rows with
    indirect_dma_start (the v1 bottleneck).
  - A second, SBUF-source transpose `dma_gather` per slot re-permutes pool
    rows into triple order, landing vec dim d on partition d: slot tiles
    [128, C].
  - Distances: u = a-b, v = u+re on DVE; the over-d reduction runs on the
    TensorEngine as lhsT=usq-block x ones matmuls whose [128, 1] outputs
    stack triples on partitions -> [128, 32] score tiles per chunk.
  - re is rebuilt per column from a [18, C] one-hot via a k=18 matmul (the
    relation table trick from v1); masks and fp16 bias-diff streams are
    per-triple functions of the index arrays and ship from the host like the
    one-hot does.
  - Final blend/relu/margin scoring on [128, 32] tiles; per-core partial sum
    [128, 1] returned; host sums / B.
"""

import os
import sys

sys.path.insert(0, "/opt/trn_rl_repo")

import numpy as np

KTRUNC = os.environ.get("KTRUNC", "0") == "1"

import concourse.bass as bass
from concourse import bacc
import concourse.tile as tile
from concourse import mybir
from concourse.bass_utils import run_bass_kernel_spmd

F16 = mybir.dt.float16
F32 = mybir.dt.float32
I16 = mybir.dt.int16

NUM_ENTITY = 100000
NUM_RELATION = 18
D = 128
B = 262144
N_CORES = 8
NB = B // N_CORES          # 32768 triples per core
P = 128
C = 4096                   # triples per chunk
NCHUNK = NB // C           # 8
NCOL = C // P              # 32  (final-layout columns per chunk)
MARGIN = 1.0

BW = 32768                 # entity bucket width (int16 idx limit)
NBUCKET = 4
BROWS = [BW, BW, BW, NUM_ENTITY - 3 * BW]      # rows per bucket slice
_KQ = int(os.environ.get("KQ", "6016"))
QUOTA = [_KQ, _KQ, _KQ, 640]                   # per-chunk gather quota (x128)
QBASE = [0, QUOTA[0], 2 * QUOTA[0], 3 * QUOTA[0]]
POOL = 3 * QUOTA[0] + QUOTA[3]                 # total pool positions
STRIPES = POOL // P

HYPONYM = (4, 6)
HYPERNYM = (3, 5)
SYNONYM = (0, 1, 13, 17)

AX_X = mybir.AxisListType.X
OP = mybir.AluOpType

NQUEUE = 1                  # SWDGE queues in use
KSTAGE = int(os.environ.get("KSTAGE", "3"))  # 1=stageA only, 2=+stageB, 3=full
KCORES = int(os.environ.get("KCORES", "8"))
HALF = 1024                 # scratch tile width (C // HALF passes)
NHALF = C // HALF


def build_bass():
    nc = bacc.Bacc(
        "TRN2", target_bir_lowering=False, debug=True, num_swdge_queues=NQUEUE
    )

    vec_t = nc.declare_dram_parameter("vec", [NUM_ENTITY, D], F16, isOutput=False)
    relT_t = nc.declare_dram_parameter("relT", [NUM_RELATION, D], F16, isOutput=False)
    ia_t = [
        nc.declare_dram_parameter(
            f"ia{q}", [NCHUNK, P, QUOTA[q] // 16], I16, isOutput=False
        )
        for q in range(NBUCKET)
    ]
    ib_t = nc.declare_dram_parameter("ib", [NCHUNK, 4, P, C // 16], I16, isOutput=False)
    oh_t = nc.declare_dram_parameter("oh", [NCHUNK, NUM_RELATION, C], F16, isOutput=False)
    bd_t = nc.declare_dram_parameter("bd", [NCHUNK, P, 3, NCOL], F16, isOutput=False)
    msk_t = nc.declare_dram_parameter("msk", [NCHUNK, P, 3, NCOL], F16, isOutput=False)
    out_t = nc.declare_dram_parameter("psum_out", [P, 1], F32, isOutput=True)

    with tile.TileContext(nc) as tc:
        with (
            tc.tile_pool(name="persist", bufs=1) as persist,
            tc.tile_pool(name="idxp", bufs=2) as idxp,
            tc.tile_pool(name="poolp", bufs=2) as poolp,
            tc.tile_pool(name="vp", bufs=2) as vp,
            tc.tile_pool(name="rep", bufs=2) as rep,
            tc.tile_pool(name="scr", bufs=2) as scr,
            tc.tile_pool(name="fin", bufs=2) as fin,
            tc.tile_pool(name="ps", bufs=2, space="PSUM") as psp,
            tc.tile_pool(name="psre", bufs=2, space="PSUM") as psrep,
        ):
            relT = persist.tile([NUM_RELATION, D], F16, name="relT")
            nc.sync.dma_start(out=relT[:], in_=relT_t[:])
            ones = persist.tile([P, 1], F16, name="ones")
            nc.vector.memset(ones[:], 1.0)
            acc = persist.tile([P, 1], F32, name="acc")
            nc.vector.memset(acc[:], 0.0)

            for c in range(NCHUNK):
                # ---- per-chunk host-prepared streams ----
                ia = [
                    idxp.tile([P, QUOTA[q] // 16], I16, name=f"ia{q}", tag=f"ia{q}")
                    for q in range(NBUCKET)
                ]
                for q in range(NBUCKET):
                    nc.sync.dma_start(out=ia[q][:], in_=ia_t[q][c])
                ib = idxp.tile([P, 4, C // 16], I16, name="ib", tag="ib")
                nc.sync.dma_start(out=ib[:], in_=ib_t[c])
                oh = idxp.tile([NUM_RELATION, C], F16, name="oh", tag="oh")
                nc.sync.dma_start(out=oh[:], in_=oh_t[c])
                bd = fin.tile([P, 3, NCOL], F16, name="bd", tag="bd")
                nc.sync.dma_start(out=bd[:], in_=bd_t[c])
                msk = fin.tile([P, 3, NCOL], F16, name="msk", tag="msk")
                nc.sync.dma_start(out=msk[:], in_=msk_t[c])

                # ---- stage A: bucket-compacted HBM gathers into the pool ----
                pool = poolp.tile([P, STRIPES, D], F16, name="pool", tag="pool")
                for q in range(NBUCKET):
                    nc.gpsimd.dma_gather(
                        out_ap=pool[:, QBASE[q] // P:(QBASE[q] + QUOTA[q]) // P, :],
                        in_ap=vec_t[q * BW:q * BW + BROWS[q], :],
                        idxs_ap=ia[q][:],
                        num_idxs=QUOTA[q],
                        num_idxs_reg=QUOTA[q],
                        elem_size=D,
                        single_packet=False,
                        queue_num=q % NQUEUE,
                    )

                # ---- stage B: SBUF transpose regather into triple order ----
                if KSTAGE < 2:
                    continue
                V = []
                for s in range(4):
                    v = vp.tile([P, 1, C], F16, name=f"V{s}", tag=f"V{s}")
                    nc.gpsimd.dma_gather(
                        out_ap=v[:, :, :],
                        in_ap=pool[:, :, :],
                        idxs_ap=ib[:, s, :],
                        num_idxs=C,
                        num_idxs_reg=C,
                        elem_size=D,
                        transpose=True,
                        sbuf_tokens_per_rank=P,
                        sbuf_free_dim_per_rank=D * 2,
                        single_packet=False,
                        queue_num=s % NQUEUE,
                    )
                    V.append(v)
                lv, rv, nlv, nrv = V

                if KSTAGE < 3:
                    continue
                # ---- re[d, j] via k=18 one-hot matmul ----
                re_sb = rep.tile([P, C], F16, name="re", tag="re")
                for w in range(C // 512):
                    ps_re = psrep.tile([P, 512], F32, name="ps_re", tag="ps_re",
                                       space="PSUM")
                    nc.tensor.matmul(
                        out=ps_re[:],
                        lhsT=relT[:],
                        rhs=oh[:, w * 512:(w + 1) * 512],
                        start=True, stop=True,
                    )
                    nc.scalar.copy(out=re_sb[:, w * 512:(w + 1) * 512], in_=ps_re[:])

                # ---- distances: 3 pairs, PE partition-reduce ----
                ST = psp.tile([P, 6, NCOL], F32, name="ST", tag="ST", space="PSUM")
                dts = []
                for k, (a, b) in enumerate(((lv, rv), (nlv, rv), (lv, nrv))):
                    Sp = ST[:, 2 * k, :]
                    Tp = ST[:, 2 * k + 1, :]
                    for h in range(NHALF):
                        hs = slice(h * HALF, (h + 1) * HALF)
                        u = scr.tile([P, HALF], F16, name="u", tag="u")
                        usq = scr.tile([P, HALF], F16, name="usq", tag="usq")
                        w_ = scr.tile([P, HALF], F16, name="w", tag="w")
                        wsq = scr.tile([P, HALF], F16, name="wsq", tag="wsq")
                        nc.vector.tensor_sub(u[:], a[:, 0, hs], b[:, 0, hs])
                        nc.vector.tensor_mul(usq[:], u[:], u[:])
                        nc.vector.tensor_add(w_[:], u[:], re_sb[:, hs])
                        nc.vector.tensor_mul(wsq[:], w_[:], w_[:])
                        for bb in range(HALF // P):
                            col = h * (HALF // P) + bb
                            nc.tensor.matmul(
                                out=Sp[:, col:col + 1],
                                lhsT=usq[:, bb * P:(bb + 1) * P],
                                rhs=ones[:],
                                start=True, stop=True,
                            )
                            nc.tensor.matmul(
                                out=Tp[:, col:col + 1],
                                lhsT=wsq[:, bb * P:(bb + 1) * P],
                                rhs=ones[:],
                                start=True, stop=True,
                            )
                    d_k = fin.tile([P, NCOL], F32, name=f"d{k}", tag=f"d{k}")
                    td_k = fin.tile([P, NCOL], F32, name=f"td{k}", tag=f"td{k}")
                    nc.scalar.sqrt(d_k[:], Sp)
                    nc.scalar.sqrt(td_k[:], Tp)
                    dts.append((d_k, td_k))

                # ---- scoring on [128, 32] tiles ----
                mh, mhy, ms = msk[:, 0, :], msk[:, 1, :], msk[:, 2, :]
                mt = fin.tile([P, NCOL], F32, name="mt", tag="mt")
                nc.vector.tensor_add(mt[:], mh, mhy)
                nc.vector.tensor_add(mt[:], mt[:], ms)
                nc.vector.tensor_scalar(
                    mt[:], mt[:], -1.0, 1.0, op0=OP.mult, op1=OP.add,
                )
                crts = []
                for k in range(3):
                    d_k, td_k = dts[k]
                    bdk = bd[:, k, :]
                    hyp = fin.tile([P, NCOL], F32, name="hyp", tag=f"hyp{k}")
                    nc.vector.tensor_sub(hyp[:], d_k[:], bdk)
                    nc.vector.tensor_scalar_max(hyp[:], hyp[:], 0.0)
                    hyr = fin.tile([P, NCOL], F32, name="hyr", tag=f"hyr{k}")
                    nc.vector.tensor_add(hyr[:], d_k[:], bdk)
                    nc.vector.tensor_scalar_max(hyr[:], hyr[:], 0.0)
                    syn = fin.tile([P, NCOL], F32, name="syn", tag=f"syn{k}")
                    nc.vector.scalar_tensor_tensor(
                        syn[:], bdk, -1.0, bdk, op0=OP.mult, op1=OP.max,
                    )
                    nc.vector.tensor_add(syn[:], syn[:], d_k[:])
                    crt = fin.tile([P, NCOL], F32, name="crt", tag=f"crt{k}")
                    nc.vector.tensor_mul(crt[:], mh, hyp[:])
                    nc.vector.tensor_mul(hyp[:], mhy, hyr[:])
                    nc.vector.tensor_add(crt[:], crt[:], hyp[:])
                    nc.vector.tensor_mul(hyp[:], ms, syn[:])
                    nc.vector.tensor_add(crt[:], crt[:], hyp[:])
                    nc.vector.tensor_mul(hyp[:], mt[:], td_k[:])
                    nc.vector.tensor_add(crt[:], crt[:], hyp[:])
                    crts.append(crt)

                q2 = fin.tile([P, NCOL], F32, name="q2", tag="q2")
                q3 = fin.tile([P, NCOL], F32, name="q3", tag="q3")
                nc.vector.tensor_sub(q2[:], crts[0][:], crts[1][:])
                nc.vector.tensor_scalar(
                    q2[:], q2[:], MARGIN, 0.0, op0=OP.add, op1=OP.max,
                )
                nc.vector.tensor_sub(q3[:], crts[0][:], crts[2][:])
                nc.vector.tensor_scalar(
                    q3[:], q3[:], MARGIN, 0.0, op0=OP.add, op1=OP.max,
                )
                nc.vector.tensor_add(q2[:], q2[:], q3[:])
                part = fin.tile([P, 1], F32, name="part", tag="part")
                nc.vector.tensor_reduce(out=part[:], in_=q2[:], axis=AX_X, op=OP.add)
                nc.vector.tensor_add(acc[:], acc[:], part[:])

            nc.sync.dma_start(out=out_t[:], in_=acc[:])

    nc.finalize()
    return nc


_NC_CACHE = {}


def _get_nc():
    if "nc" not in _NC_CACHE:
        _NC_CACHE["nc"] = build_bass()
    return _NC_CACHE["nc"]


def _wrap16(vals, pad_to):
    """[n] -> [128, pad_to//16] int16 tile: position i at [i%16, i//16],
    replicated across all 8 16-partition groups (the SWDGE core pair reads
    its own group)."""
    buf = np.zeros(pad_to, dtype=np.int16)
    buf[: len(vals)] = vals
    return np.tile(buf.reshape(pad_to // 16, 16).T, (8, 1))


def _prep_core(ids, rel, biases):
    """Build per-core host streams. ids: [4, NB] int64 slot-major (l, r, nl, nr),
    rel: [NB], biases: [NUM_ENTITY] f32."""
    ia = [np.zeros((NCHUNK, P, QUOTA[q] // 16), dtype=np.int16) for q in range(NBUCKET)]
    ib = np.zeros((NCHUNK, 4, P, C // 16), dtype=np.int16)
    oh = np.zeros((NCHUNK, NUM_RELATION, C), dtype=np.float16)
    bdv = np.zeros((NCHUNK, P, 3, NCOL), dtype=np.float16)
    mskv = np.zeros((NCHUNK, P, 3, NCOL), dtype=np.float16)

    rids = np.arange(NUM_RELATION)
    is_h = np.isin(rids, HYPONYM).astype(np.float16)
    is_hy = np.isin(rids, HYPERNYM).astype(np.float16)
    is_s = np.isin(rids, SYNONYM).astype(np.float16)

    for c in range(NCHUNK):
        sl = slice(c * C, (c + 1) * C)
        ents = ids[:, sl].reshape(-1)            # [4*C] slot-major
        bucket = ents // BW
        poolpos = np.empty(4 * C, dtype=np.int64)
        for q in range(NBUCKET):
            sel = np.nonzero(bucket == q)[0]
            if KTRUNC:
                sel = sel[:QUOTA[q]]
            nq = len(sel)
            assert nq <= QUOTA[q], f"bucket {q} overflow: {nq} > {QUOTA[q]}"
            ia[q][c] = _wrap16((ents[sel] - q * BW).astype(np.int16), QUOTA[q])
            poolpos[sel] = QBASE[q] + np.arange(nq)
        pp = poolpos.reshape(4, C)
        for s in range(4):
            ib[c, s] = _wrap16(pp[s].astype(np.int16), C)

        rc = rel[sl]
        oh[c][rc, np.arange(C)] = 1.0

        # final layout: triple j -> (partition j%128, col j//128)
        def fin_layout(v):
            return v.reshape(NCOL, P).T.astype(np.float16)

        bl, br, bnl, bnr = (biases[ids[s, sl]] for s in range(4))
        bdv[c, :, 0, :] = fin_layout(bl - br)
        bdv[c, :, 1, :] = fin_layout(bnl - br)
        bdv[c, :, 2, :] = fin_layout(bl - bnr)
        mskv[c, :, 0, :] = fin_layout(is_h[rc])
        mskv[c, :, 1, :] = fin_layout(is_hy[rc])
        mskv[c, :, 2, :] = fin_layout(is_s[rc])
    return ia, ib, oh, bdv, mskv


def _prep_inputs(inputs):
    vec = np.ascontiguousarray(
        np.asarray(inputs["predVec"], dtype=np.float32).astype(np.float16)
    )
    relT = np.asarray(inputs["relEmb"], dtype=np.float32).astype(np.float16)
    biases = np.asarray(inputs["predBias"], dtype=np.float32).reshape(NUM_ENTITY)

    li = np.asarray(inputs["leftEnIndices"], dtype=np.int64)
    ri = np.asarray(inputs["rightEnIndices"], dtype=np.int64)
    nli = np.asarray(inputs["negLeftEnIndices"], dtype=np.int64)
    nri = np.asarray(inputs["negRightEnIndices"], dtype=np.int64)
    rel = np.asarray(inputs["relIndices"], dtype=np.int64)

    in_maps = []
    for core in range(N_CORES):
        sl = slice(core * NB, (core + 1) * NB)
        ids = np.stack([li[sl], ri[sl], nli[sl], nri[sl]])
        ia, ib, oh, bdv, mskv = _prep_core(ids, rel[sl], biases)
        m = {"vec": vec, "relT": relT, "ib": ib, "oh": oh, "bd": bdv, "msk": mskv}
        for q in range(NBUCKET):
            m[f"ia{q}"] = ia[q]
        in_maps.append(m)
    return in_maps


def run(inputs, trace=False):
    nc = _get_nc()
    in_maps = _prep_inputs(inputs)
    res = run_bass_kernel_spmd(nc, in_maps[:KCORES], core_ids=list(range(KCORES)), trace=trace)
    total = sum(float(r["psum_out"].astype(np.float64).sum()) for r in res.results)
    out = np.float32(total / B)
    return np.asarray(out, dtype=np.float32), res


def kernel(**inputs) -> np.ndarray:
    out, _ = run(inputs, trace=False)
    return out
